# revision 1
# baseline (speedup 1.0000x reference)
"""Chamfer distance + F1 kernel for Trainium2 (8 NeuronCores).

Strategy (B=4 batches, N=M=8192 points, 3D):
  - core c handles batch b = c//2, row-half h = c%2 of xyz1 (4096 rows).
  - PE computes the scaled squared-distance block 4096*d[p,f] in ONE fp16
    matmul pass using an augmented K=13 contraction built on the host:
    each fp32 operand is split into fp16 hi+lo parts so the result is
    accurate to ~1e-6 while streaming at full bf16/fp16 PE rate.
  - ACT converts PSUM fp32 -> SBUF fp16 (with Relu), enabling DVE 2x mode.
  - DVE computes per-row mins (dist1) via a fused custom DVE op
    (out=min(lo,hi), accum_out=min-fold), and a running elementwise min
    across row-tiles (col-min accumulator M, fp16 tensor_tensor at 2x).
  - PE transposes M, DVE reduces -> per-column partial mins (dist2 half).
  - Host combines the two halves per batch and computes cd_p/cd_t/f1 on
    the 8192-element min vectors (0.01% of the FLOPs).
"""

import sys

if "/opt/trn_rl_repo" not in sys.path:
    sys.path.insert(0, "/opt/trn_rl_repo")

from contextlib import ExitStack

import numpy as np

import concourse.tile as tile
import concourse.dve_ops as dve_ops
from concourse import bacc, bass_isa, mybir
from concourse.bass_utils import run_bass_kernel_spmd
from concourse.dve_spec import C0, AluOp, Spec, Src0, Src1, lower, minn
from concourse.dve_uop import DveOpSpec

F16 = mybir.dt.float16
F32 = mybir.dt.float32
MIN = mybir.AluOpType.min
AXX = mybir.AxisListType.X

SCALE = 24.0  # coordinate prescale; distances come out scaled by SCALE**2
# (max pairwise sq-dist for these inputs is ~92; 92*24^2 = 53k < fp16 max)
DSCALE = SCALE * SCALE
F1_THRESHOLD = 1e-4

N_CORES = 8
K_AUG = 13  # 9 coord-product rows + 2 sq1 rows + 2 sq2 rows
USE_TS_ROWMIN = False  # rowmin via builtin tensor_scalar min-accum (1x only)
USE_MINMIN_2X = False  # hand-authored 2x uop program for the MINMIN rowmin

_RealCDA = bass_isa.InstCustomDveAnt


def _cda_perf(*a, **kw):
    """bass.py constructs InstCustomDveAnt via module attr; force perf_max=1
    (2x_1PORT opt-in, instruction byte 36[7:6]) for the MINMIN op."""
    inst = _RealCDA(*a, **kw)
    if USE_MINMIN_2X and kw.get("op_name") == "MINMIN_REDUCE_ANT":
        inst.perf_max = 1
    return inst


bass_isa.InstCustomDveAnt = _cda_perf


def _split16(v):
    """Split fp32 array into fp16 hi + lo so hi+lo ~= v to ~2^-22 rel."""
    hi = v.astype(np.float16)
    lo = (v - hi.astype(np.float32)).astype(np.float16)
    return hi, lo


def _prep_core(xyz1_half, xyz2_full):
    """Build the augmented fp16 operands for one core.

    Returns lhsT [13, n_rows] (stationary, xyz1 side) and
    rhs [13, n_cols] (moving, xyz2 side) such that
    sum_k lhsT[k,p] * rhs[k,f] ~= DSCALE * ||xyz1[p] - xyz2[f]||^2.
    """
    n_rows = xyz1_half.shape[0]
    n_cols = xyz2_full.shape[0]
    v1 = (-2.0 * SCALE) * xyz1_half.astype(np.float32)  # [n_rows, 3]
    w2 = SCALE * xyz2_full.astype(np.float32)  # [n_cols, 3]
    h1, l1 = _split16(v1)
    h2, l2 = _split16(w2)

    s1 = (SCALE * xyz1_half.astype(np.float32)) ** 2
    s1q = s1.sum(axis=1) * 0.25  # DSCALE*sq1 / 4
    s2q = ((w2.astype(np.float64) ** 2).sum(axis=1) * 0.25).astype(np.float32)
    s1h, s1l = _split16(s1q)
    s2h, s2l = _split16(s2q)

    lhsT = np.empty((K_AUG, n_rows), np.float16)
    rhs = np.empty((K_AUG, n_cols), np.float16)
    for c in range(3):
        lhsT[3 * c + 0] = h1[:, c]
        lhsT[3 * c + 1] = h1[:, c]
        lhsT[3 * c + 2] = l1[:, c]
        rhs[3 * c + 0] = h2[:, c]
        rhs[3 * c + 1] = l2[:, c]
        rhs[3 * c + 2] = h2[:, c]
    lhsT[9] = s1h
    lhsT[10] = s1l
    rhs[9] = np.float16(4.0)
    rhs[10] = np.float16(4.0)
    lhsT[11] = np.float16(4.0)
    lhsT[12] = np.float16(4.0)
    rhs[11] = s2h
    rhs[12] = s2l
    return lhsT, rhs


def _build_minmin_2x(uops_1x):
    """Hand-authored 2x_1PORT uop program for MINMIN (no stock accum op runs
    above 1x; this processes 2 packed fp16 pairs per port per cycle).

    steady-state, per cycle:
      lanes: 1=SRC_0 2=SRC_1 3=SRC_0_HI 4=SRC_1_HI
      s0: lo = MIN(SRC_0, SRC_1); delay lanes carry the HI pair forward
      s1: hi = MIN(SRC_0_HI, SRC_1_HI); delay0 captures s0's lo (realign)
      s2: merged = MIN(hi[curr], lo[delay0])
      s3: acc = MIN(merged[curr], acc[self])  (alu_out_a = accumulator)
      s4-7: hold.  out tensor gets scratch values; only accum_out is used.
    """
    from copy import deepcopy

    from concourse.dve_uop import (
        AluInp, DelayInp, InpSel, OutPath, OutSel, Trigger,
        UopConfig, UopDpConfig,
    )
    from concourse.dve_uop import AluOp as UAluOp

    PD = DelayInp.PREV_DELAY
    PAO = DelayInp.PREV_ALU_OUT

    def dp(op, s0, s1, a=0):
        return UopDpConfig(
            op=op, alu_src0=s0, alu_src1=s1,
            delay=[PD, PD, PD, PD, PAO, PAO, PAO],
            alu_out_enable=1, swap_enable=0,
            alu_out_a_enable=a, alu_out_b_enable=0,
            delay_enable=[1, 0, 0, 0, 0, 0, 0],
            idx0_sel=0, idx1_sel=0,
        )

    s0 = dp(UAluOp.MIN, AluInp.PREV_DELAY_0, AluInp.PREV_DELAY_1)
    s0.delay_enable = [1, 1, 1, 1, 0, 0, 0]
    s1 = dp(UAluOp.MIN, AluInp.PREV_DELAY_2, AluInp.PREV_DELAY_3)
    s1.delay = [PAO, PD, PD, PD, PAO, PAO, PAO]
    stages = [
        s0,
        s1,
        # s2: PREV_ALU_OUT = s1's hi (prev block), PREV_DELAY_0 = lo
        dp(UAluOp.MIN, AluInp.PREV_ALU_OUT, AluInp.PREV_DELAY_0),
        # s3: CURR_ALU_OUT = own flop = the accumulator (seeded with C0)
        dp(UAluOp.MIN, AluInp.CURR_ALU_OUT, AluInp.PREV_ALU_OUT, a=1),
        dp(UAluOp.BYPASS, AluInp.PREV_ALU_OUT, AluInp.PREV_ALU_OUT, a=1),
        dp(UAluOp.BYPASS, AluInp.PREV_ALU_OUT, AluInp.PREV_ALU_OUT, a=1),
        dp(UAluOp.BYPASS, AluInp.PREV_ALU_OUT, AluInp.PREV_ALU_OUT, a=1),
        dp(UAluOp.BYPASS, AluInp.PREV_ALU_OUT, AluInp.PREV_ALU_OUT, a=1),
    ]

    def seed_dp():
        # seed token (1 cycle): C0 enters on input lane 3, rides delay
        # lane 2 to stage 3 which loads the accumulator flop; stage 0
        # bypasses C0 so stage 1's delay lane 0 (the merge operand in the
        # steady state) also starts at C0 instead of stale garbage.
        st0 = dp(UAluOp.BYPASS, AluInp.PREV_DELAY_2, AluInp.PREV_DELAY_2)
        st0.delay_enable = [1, 1, 1, 0, 0, 0, 0]
        st1 = dp(UAluOp.BYPASS, AluInp.PREV_ALU_OUT, AluInp.PREV_ALU_OUT)
        st1.delay = [PAO, PD, PD, PD, PAO, PAO, PAO]
        st1.delay_enable = [1, 0, 1, 0, 0, 0, 0]
        st2 = dp(UAluOp.BYPASS, AluInp.PREV_ALU_OUT, AluInp.PREV_ALU_OUT)
        st2.delay_enable = [1, 0, 1, 0, 0, 0, 0]
        st3 = dp(UAluOp.BYPASS, AluInp.PREV_DELAY_2, AluInp.PREV_DELAY_2, a=1)
        return [st0, st1, st2, st3] + [
            dp(UAluOp.BYPASS, AluInp.PREV_ALU_OUT, AluInp.PREV_ALU_OUT, a=1)
            for _ in range(4)
        ]
    steady = UopConfig(
        inp=[InpSel.ZERO, InpSel.SRC_0, InpSel.SRC_1, InpSel.SRC_0_HI,
             InpSel.SRC_1_HI, InpSel.ZERO, InpSel.ZERO, InpSel.ZERO],
        inp_enable=[0, 1, 1, 1, 1, 0, 0, 0],
        out={OutPath.WR0_LO: OutSel.ALU_OUT, OutPath.WR0_HI: OutSel.DELAY_0,
             OutPath.WR1_LO: OutSel.ALU_OUT, OutPath.WR1_HI: OutSel.ALU_OUT},
        out_enable={OutPath.WR0_LO: 1, OutPath.WR0_HI: 1,
                    OutPath.WR1_LO: 0, OutPath.WR1_HI: 0},
        out_last_subdim_enable=0,
        force_two_data_zero=0, force_two_data_one=0,
        require_inp0=1, require_inp1=1,
        repeat_count=0,
        trigger=(Trigger.SRC_TENSOR_DONE, Trigger.NONE, Trigger.NONE),
        next_uop=(0, 0, 0),
        inc_parameter_index=0, enable_rev_ops=0,
        match_mask=0, valid_match=0, replace_on_match=0, clear_match=0,
        write_predicate_select=0, write_predicate_enable=0,
        delay_shift8=0, index_increment=0, index_clear=0,
        accum_enabled=1, v4={},
        datapath_config=stages,
    )
    seed = deepcopy(uops_1x[0])
    seed.datapath_config = seed_dp()
    return [seed, steady]


def _register_minmin_reduce():
    """Custom DVE op: out = min(in0, in1); accum_out = min-fold(out).

    Replaces a 5-op fold tree for the per-row min: one pass over the two
    halves of a tile yields the full row min in accum_out. Registered
    dynamically in dve_ops.OPS (the per-NEFF uop table is generated from
    the ops actually used at compile time). A hand-authored 2x_1PORT uop
    variant is injected via the compile cache; the emitted instruction
    opts in with perf_max=1.
    """
    name = "MINMIN_REDUCE_ANT"
    if name in dve_ops._SUB_OPCODE_FOR_NAME:
        return next(op for op in dve_ops.OPS if op.name == name)

    def _ref(in0, in1, c0, c1, c2):
        out = np.minimum(np.asarray(in0, np.float32), np.asarray(in1, np.float32))
        acc = out.reshape(out.shape[0], -1).min(axis=-1, keepdims=True)
        acc = np.minimum(acc, c0)
        return out, acc

    spec = Spec(body=minn(Src0, Src1), accum=AluOp.MIN, accum_init=C0,
                reference=_ref)
    row = max(dve_ops._SUB_OPCODE_FOR_NAME.values()) + 1
    u1 = lower(spec, ver="v3")
    s3 = DveOpSpec(name=name, opcode=row, uops=u1,
                   uops_2x=_build_minmin_2x(u1) if USE_MINMIN_2X else None,
                   rd1_en=True, perf_max=1 if USE_MINMIN_2X else 0)
    s3.validate("v3")
    shas = {"v3": s3.sha("v3")}
    try:
        u1v4 = lower(spec, ver="v4")
        s4 = DveOpSpec(name=name, opcode=row, uops=u1v4, rd1_en=True)
        shas["v4"] = s4.sha("v4")
        dve_ops._COMPILE_CACHE[(name, "v4")] = s4
    except Exception:
        pass
    op = dve_ops.DveOp(name, spec, subdim=False, uops_sha=shas)
    dve_ops._COMPILE_CACHE[(name, "v3")] = s3
    dve_ops.OPS.append(op)
    dve_ops.CUSTOM_DVE_SPECS[name] = spec
    dve_ops._SUB_OPCODE_FOR_NAME[name] = row
    return op


def build_program(n_rows=4096, n_cols=8192):
    """Build + compile the per-core Bass program (same program on all cores)."""
    ROWT = n_rows // 128  # row tiles
    CG = min(2048, n_cols)  # ACT convert granule (4 PSUM banks)
    NG = n_cols // CG  # granules per row tile
    NMM = CG // 512  # matmuls per granule
    NB = n_cols // 128  # 128-col blocks for the transpose tail
    PER = min(16, NB)  # transpose blocks per PSUM tile

    MINMIN = _register_minmin_reduce()
    nc = bacc.Bacc("TRN2", target_bir_lowering=False, debug=False,
                   num_devices=N_CORES)
    lhsT_d = nc.dram_tensor("lhsT", [K_AUG, n_rows], F16, kind="ExternalInput").ap()
    rhs_d = nc.dram_tensor("rhs", [K_AUG, n_cols], F16, kind="ExternalInput").ap()
    id_d = nc.dram_tensor("ident", [128, 128], F16, kind="ExternalInput").ap()
    out1_d = nc.dram_tensor("out1", [128, ROWT], F32, kind="ExternalOutput").ap()
    out2_d = nc.dram_tensor("out2", [128, NB], F32, kind="ExternalOutput").ap()

    with tile.TileContext(nc) as tc, ExitStack() as ctx:
        const = ctx.enter_context(tc.tile_pool(name="const", bufs=1))
        d16p = ctx.enter_context(tc.tile_pool(name="d16", bufs=3))
        mp = ctx.enter_context(tc.tile_pool(name="m875", bufs=1))
        treep = ctx.enter_context(tc.tile_pool(name="tree", bufs=2))
        outp = ctx.enter_context(tc.tile_pool(name="outs", bufs=1))
        psp = ctx.enter_context(tc.tile_pool(name="ps", bufs=2, space="PSUM"))

        w_sb = const.tile([K_AUG, n_rows], F16)
        nc.sync.dma_start(w_sb[:], lhsT_d)
        r_sb = const.tile([K_AUG, n_cols], F16)
        # chunked so the first matmuls only wait on their own slice; the
        # leading chunks are small to light up the PE->ACT->DVE pipe early
        s = 0
        for w in [512, 512, 1024] + [CG] * (n_cols // CG):
            if s >= n_cols:
                break
            w = min(w, n_cols - s)
            nc.sync.dma_start(r_sb[:, s:s + w], rhs_d[:, s:s + w])
            s += w
        id_sb = const.tile([128, 128], F16)
        nc.sync.dma_start(id_sb[:], id_d)

        M = mp.tile([128, n_cols], F16)
        R = outp.tile([128, ROWT], F32)
        C = outp.tile([128, NB], F32)

        def granule_widths(t):
            # tile 0 leads with small granules so the PE->ACT->DVE pipeline
            # starts as early as possible; steady state uses CG-wide granules
            widths, s = [], 0
            lead = [512, 512, 1024] if t == 0 and n_cols >= 4 * CG else []
            for w in lead + [CG] * (n_cols // 512):
                if s >= n_cols:
                    break
                w = min(w, n_cols - s)
                widths.append(w)
                s += w
            return widths

        for t in range(ROWT):
            # tile 0 converts straight into the col-min accumulator M —
            # its "running min" against nothing is just itself
            d16 = M if t == 0 else d16p.tile([128, n_cols], F16, tag="d16")
            gs = 0
            for w in granule_widths(t):
                ps = psp.tile([128, w], F32, tag="ps")
                for j in range(w // 512):
                    nc.tensor.matmul(
                        ps[:, 512 * j:512 * (j + 1)],
                        w_sb[:, 128 * t:128 * (t + 1)],
                        r_sb[:, gs + 512 * j:gs + 512 * (j + 1)],
                        start=True, stop=True,
                    )
                nc.scalar.activation(
                    d16[:, gs:gs + w], ps[:],
                    mybir.ActivationFunctionType.Relu,
                )
                # early tiles: granule-wise col-min so DVE consumes each
                # converted granule as it lands during ramp-up
                if 0 < t <= 2:
                    nc.vector.tensor_tensor(M[:, gs:gs + w], M[:, gs:gs + w],
                                            d16[:, gs:gs + w], op=MIN)
                gs += w
            if t > 2:
                nc.vector.tensor_tensor(M[:], M[:], d16[:], op=MIN)
            if USE_TS_ROWMIN:
                # row-min: builtin tensor_scalar with min-accum. Single-src
                # fp16 SBUF step-1 => eligible for 4x perf mode (2 ports x
                # 2-packed reads). out = min(d16, 65504) = d16 (scratch);
                # accum_out = min-reduce of the stream = the row min.
                u = treep.tile([128, n_cols], F16, tag="mm_scratch")
                nc.vector.tensor_scalar(
                    out=u[:], in0=d16[:], scalar1=65504.0, scalar2=None,
                    op0=MIN, op1=MIN, accum_out=R[:, t:t + 1],
                )
            else:
                # row-min: one fused custom op over the two tile halves
                half = n_cols // 2
                u = treep.tile([128, half], F16, tag="mm_scratch")
                nc.vector._custom_dve(
                    MINMIN, out=u[:], in0=d16[:, 0:half], in1=d16[:, half:n_cols],
                    s0=65504.0, accum_out=R[:, t:t + 1],
                )

        # column-min of M across its 128 partitions: PE-transpose 128-col
        # blocks into PSUM (manual start/stop: 8 fp16 blocks share a bank),
        # then reduce along the transposed free dim.
        for q in range(NB // PER):
            psT = psp.tile([128, PER * 128], F16, tag="ps")
            for j in range(PER):
                blk = q * PER + j
                nc.tensor.matmul(
                    psT[:, 128 * j:128 * (j + 1)],
                    M[:, 128 * blk:128 * (blk + 1)],
                    id_sb[:],
                    is_transpose=True,
                    start=(j % 8 == 0), stop=(j % 8 == 7),
                )
            nc.vector.tensor_reduce(
                C[:, PER * q:PER * (q + 1)],
                psT[:].rearrange("p (b c) -> p b c", c=128),
                axis=AXX, op=MIN,
            )

        nc.sync.dma_start(out1_d, R[:])
        nc.sync.dma_start(out2_d, C[:])

    nc.compile()
    return nc


_CACHE = {}


def _get_program(n_rows, n_cols):
    key = (n_rows, n_cols)
    if key not in _CACHE:
        _CACHE[key] = build_program(n_rows, n_cols)
    return _CACHE[key]


def run_device(xyz1, xyz2, trace=False):
    """Run the 8-core SPMD program; returns (dist1 [B,N], dist2 [B,M], results)."""
    xyz1 = np.asarray(xyz1)
    xyz2 = np.asarray(xyz2)
    B, N, _ = xyz1.shape
    M = xyz2.shape[1]
    halves = N_CORES // B  # row-halves per batch (2)
    n_rows = N // halves
    nc = _get_program(n_rows, M)

    ident = np.eye(128, dtype=np.float16)
    in_maps = []
    for c in range(N_CORES):
        b, h = divmod(c, halves)
        lhsT, rhs = _prep_core(
            xyz1[b, h * n_rows:(h + 1) * n_rows], xyz2[b])
        in_maps.append({"lhsT": lhsT, "rhs": rhs, "ident": ident})

    res = run_bass_kernel_spmd(nc, in_maps, list(range(N_CORES)), trace=trace)

    dist1 = np.empty((B, N), np.float64)
    dist2p = np.empty((B, halves, M), np.float64)
    for c in range(N_CORES):
        b, h = divmod(c, halves)
        o1 = res.results[c]["out1"].astype(np.float64)  # [128, ROWT]
        o2 = res.results[c]["out2"].astype(np.float64)  # [128, NB]
        dist1[b, h * n_rows:(h + 1) * n_rows] = o1.T.reshape(-1)
        dist2p[b, h] = o2.T.reshape(-1)
    dist1 /= DSCALE
    dist2 = dist2p.min(axis=1) / DSCALE
    return dist1, dist2, res


def _finalize(dist1, dist2):
    cd_p = (np.sqrt(dist1).mean(axis=1) + np.sqrt(dist2).mean(axis=1)) / 2.0
    cd_t = dist1.mean(axis=1) + dist2.mean(axis=1)
    p1 = (dist1 < F1_THRESHOLD).mean(axis=1)
    p2 = (dist2 < F1_THRESHOLD).mean(axis=1)
    denom = p1 + p2
    f1 = np.where(denom > 0, 2.0 * p1 * p2 / np.where(denom > 0, denom, 1.0), 0.0)
    return (cd_p.astype(np.float32), cd_t.astype(np.float32),
            f1.astype(np.float32))


def kernel(xyz1, xyz2):
    dist1, dist2, _ = run_device(xyz1, xyz2, trace=False)
    return _finalize(dist1, dist2)



# revision 5
# speedup vs baseline: 5.0191x; 5.0191x over previous
"""Chamfer distance + F1 kernel for Trainium2 (8 NeuronCores).

Banded-KNN strategy (B=4 batches, N=M=8192 points, 3D):
  - Host sorts each batch's clouds by radius r=||p||.  Since radius is
    1-Lipschitz (|r_x - r_y| <= ||x-y||), a point's nearest neighbour is
    radially close whenever it is close in space, so a narrow band around
    the radius-sorted diagonal contains the NN for all but a handful of
    isolated points.
  - core c handles batch b = c//2, sorted-row-half h = c%2 (4096 rows).
    Each 128-row tile t computes scaled squared distances only against a
    static W-wide window of radius-sorted xyz2 (window slides 128 cols per
    tile).  The host hands each core a pre-shifted, padded slice of sorted
    xyz2 so the compiled program is identical across cores (SPMD).
  - PE computes the W-wide block in fp16 via the augmented K=13 contraction
    (fp32 operands hi/lo-split into fp16), ACT converts PSUM fp32 -> SBUF
    fp16 with Relu, DVE does the banded row-min (fused custom MINMIN op:
    out=min(lo,hi), accum=min-fold) and a sliding col-min accumulator M
    (fp16 tensor_tensor at 2x).  PE transposes M, DVE reduces -> col mins.
  - Host patch: points whose banded min exceeds the window's radial
    coverage (provable criterion) get exact numpy rows (~30/batch-side),
    making the result exact up to fp16 rounding.  cd/f1 host finalize.
"""

import sys

if "/opt/trn_rl_repo" not in sys.path:
    sys.path.insert(0, "/opt/trn_rl_repo")

from contextlib import ExitStack

import numpy as np

import concourse.tile as tile
import concourse.dve_ops as dve_ops
from concourse import bacc, bass_isa, mybir
from concourse.bass_utils import run_bass_kernel_spmd
from concourse.dve_spec import C0, AluOp, Spec, Src0, Src1, lower, minn
from concourse.dve_uop import DveOpSpec

F16 = mybir.dt.float16
F32 = mybir.dt.float32
MIN = mybir.AluOpType.min
AXX = mybir.AxisListType.X

SCALE = 24.0  # coordinate prescale; distances come out scaled by SCALE**2
DSCALE = SCALE * SCALE
F1_THRESHOLD = 1e-4
PAD_DIST = 60000.0  # scaled distance of padding columns (> max real ~53k)

N_CORES = 8
K_AUG = 13  # 9 coord-product rows + 2 sq1 rows + 2 sq2 rows
BAND_W = 1024  # band width (columns per row-tile window)
USE_MINMIN_2X = False

_RealCDA = bass_isa.InstCustomDveAnt


def _cda_perf(*a, **kw):
    inst = _RealCDA(*a, **kw)
    if USE_MINMIN_2X and kw.get("op_name") == "MINMIN_REDUCE_ANT":
        inst.perf_max = 1
    return inst


bass_isa.InstCustomDveAnt = _cda_perf


def _split16(v):
    """Split fp32 array into fp16 hi + lo so hi+lo ~= v to ~2^-22 rel."""
    hi = v.astype(np.float16)
    lo = (v - hi.astype(np.float32)).astype(np.float16)
    return hi, lo


def _aug_rows(xyz, side):
    """Augmented fp16 operand rows [K_AUG, n] for one cloud.

    side='lhs': rows for xyz1 (stationary), side='rhs': rows for xyz2
    (moving), such that sum_k lhsT[k,p]*rhs[k,f] ~= DSCALE*||x1_p - x2_f||^2.
    """
    n = xyz.shape[0]
    out = np.empty((K_AUG, n), np.float16)
    if side == "lhs":
        v = (-2.0 * SCALE) * xyz.astype(np.float32)
        h, l = _split16(v)
        sq = ((SCALE * xyz.astype(np.float32)) ** 2).sum(axis=1) * 0.25
        sh, sl = _split16(sq)
        for c in range(3):
            out[3 * c + 0] = h[:, c]
            out[3 * c + 1] = h[:, c]
            out[3 * c + 2] = l[:, c]
        out[9] = sh
        out[10] = sl
        out[11] = np.float16(4.0)
        out[12] = np.float16(4.0)
    else:
        w = SCALE * xyz.astype(np.float32)
        h, l = _split16(w)
        sq = ((w.astype(np.float64) ** 2).sum(axis=1) * 0.25).astype(np.float32)
        sh, sl = _split16(sq)
        for c in range(3):
            out[3 * c + 0] = h[:, c]
            out[3 * c + 1] = l[:, c]
            out[3 * c + 2] = h[:, c]
        out[9] = np.float16(4.0)
        out[10] = np.float16(4.0)
        out[11] = sh
        out[12] = sl
    return out


def _register_minmin_reduce():
    """Custom DVE op: out = min(in0, in1); accum_out = min-fold(out)."""
    name = "MINMIN_REDUCE_ANT"
    if name in dve_ops._SUB_OPCODE_FOR_NAME:
        return next(op for op in dve_ops.OPS if op.name == name)

    def _ref(in0, in1, c0, c1, c2):
        out = np.minimum(np.asarray(in0, np.float32), np.asarray(in1, np.float32))
        acc = out.reshape(out.shape[0], -1).min(axis=-1, keepdims=True)
        acc = np.minimum(acc, c0)
        return out, acc

    spec = Spec(body=minn(Src0, Src1), accum=AluOp.MIN, accum_init=C0,
                reference=_ref)
    row = max(dve_ops._SUB_OPCODE_FOR_NAME.values()) + 1
    u1 = lower(spec, ver="v3")
    s3 = DveOpSpec(name=name, opcode=row, uops=u1, rd1_en=True, perf_max=0)
    s3.validate("v3")
    shas = {"v3": s3.sha("v3")}
    try:
        u1v4 = lower(spec, ver="v4")
        s4 = DveOpSpec(name=name, opcode=row, uops=u1v4, rd1_en=True)
        shas["v4"] = s4.sha("v4")
        dve_ops._COMPILE_CACHE[(name, "v4")] = s4
    except Exception:
        pass
    op = dve_ops.DveOp(name, spec, subdim=False, uops_sha=shas)
    dve_ops._COMPILE_CACHE[(name, "v3")] = s3
    dve_ops.OPS.append(op)
    dve_ops.CUSTOM_DVE_SPECS[name] = spec
    dve_ops._SUB_OPCODE_FOR_NAME[name] = row
    return op


def build_program(n_rows=4096, W=BAND_W):
    """Per-core banded program (SPMD-identical across cores).

    Local column space C = n_rows + W; tile t (128 rows) sees window
    [128*t, 128*t + W).  The host pre-shifts/pads each core's rhs so this
    static window pattern is centred on the tile's radius range.
    """
    ROWT = n_rows // 128
    C = n_rows + W
    NB = C // 128  # 128-col blocks for the transpose tail
    PER = 16  # transpose blocks per PSUM tile
    NMM = W // 512  # matmuls per tile (one PSUM bank each)

    MINMIN = _register_minmin_reduce()
    nc = bacc.Bacc("TRN2", target_bir_lowering=False, debug=False,
                   num_devices=N_CORES)
    lhsT_d = nc.dram_tensor("lhsT", [K_AUG, n_rows], F16, kind="ExternalInput").ap()
    rhs_d = nc.dram_tensor("rhs", [K_AUG, C], F16, kind="ExternalInput").ap()
    id_d = nc.dram_tensor("ident", [128, 128], F16, kind="ExternalInput").ap()
    out1_d = nc.dram_tensor("out1", [128, ROWT], F32, kind="ExternalOutput").ap()
    out2_d = nc.dram_tensor("out2", [128, NB], F32, kind="ExternalOutput").ap()

    with tile.TileContext(nc) as tc, ExitStack() as ctx:
        const = ctx.enter_context(tc.tile_pool(name="const", bufs=1))
        d16p = ctx.enter_context(tc.tile_pool(name="d16", bufs=4))
        mp = ctx.enter_context(tc.tile_pool(name="macc", bufs=1))
        treep = ctx.enter_context(tc.tile_pool(name="tree", bufs=2))
        outp = ctx.enter_context(tc.tile_pool(name="outs", bufs=1))
        psp = ctx.enter_context(tc.tile_pool(name="ps", bufs=2, space="PSUM"))
        pstp = ctx.enter_context(tc.tile_pool(name="psT", bufs=2, space="PSUM"))

        w_sb = const.tile([K_AUG, n_rows], F16)
        nc.sync.dma_start(w_sb[:], lhsT_d)
        r_sb = const.tile([K_AUG, C], F16)
        # chunked so the first matmuls only wait on their own slice
        s = 0
        for w in [W + 256] + [1024] * (C // 1024):
            if s >= C:
                break
            w = min(w, C - s)
            nc.sync.dma_start(r_sb[:, s:s + w], rhs_d[:, s:s + w])
            s += w
        id_sb = const.tile([128, 128], F16)
        nc.sync.dma_start(id_sb[:], id_d)

        M = mp.tile([128, C], F16)
        R = outp.tile([128, ROWT], F32)
        C2 = outp.tile([128, NB], F32)

        # init the col-min accumulator to "infinity" (padding distance)
        nc.vector.memset(M[:], PAD_DIST)

        for t in range(ROWT):
            lo = 128 * t
            ps = psp.tile([128, W], F32, tag="ps")
            for j in range(NMM):
                nc.tensor.matmul(
                    ps[:, 512 * j:512 * (j + 1)],
                    w_sb[:, 128 * t:128 * (t + 1)],
                    r_sb[:, lo + 512 * j:lo + 512 * (j + 1)],
                    start=True, stop=True,
                )
            d16 = d16p.tile([128, W], F16, tag="d16")
            nc.scalar.activation(
                d16[:], ps[:], mybir.ActivationFunctionType.Relu,
            )
            # sliding col-min accumulate over this tile's window
            nc.vector.tensor_tensor(M[:, lo:lo + W], M[:, lo:lo + W],
                                    d16[:], op=MIN)
            # banded row-min via the fused custom op over the two halves
            half = W // 2
            u = treep.tile([128, half], F16, tag="mm_scratch")
            nc.vector._custom_dve(
                MINMIN, out=u[:], in0=d16[:, 0:half], in1=d16[:, half:W],
                s0=PAD_DIST, accum_out=R[:, t:t + 1],
            )

        # column-min of M across its 128 partitions: PE-transpose 128-col
        # blocks into PSUM, then reduce along the transposed free dim.
        q = 0
        while q * PER < NB:
            nblk = min(PER, NB - q * PER)
            psT = pstp.tile([128, PER * 128], F16, tag="psT")
            for j in range(nblk):
                blk = q * PER + j
                nc.tensor.matmul(
                    psT[:, 128 * j:128 * (j + 1)],
                    M[:, 128 * blk:128 * (blk + 1)],
                    id_sb[:],
                    is_transpose=True,
                    start=(j % 8 == 0), stop=(j % 8 == 7 or j == nblk - 1),
                )
            nc.vector.tensor_reduce(
                C2[:, PER * q:PER * q + nblk],
                psT[:, 0:nblk * 128].rearrange("p (b c) -> p b c", c=128),
                axis=AXX, op=MIN,
            )
            q += 1

        nc.sync.dma_start(out1_d, R[:])
        nc.sync.dma_start(out2_d, C2[:])

    nc.compile()
    return nc


_CACHE = {}


def _get_program(n_rows, W):
    key = (n_rows, W)
    if key not in _CACHE:
        _CACHE[key] = build_program(n_rows, W)
    return _CACHE[key]


def run_device(xyz1, xyz2, trace=False):
    """Run the 8-core SPMD banded program; returns (dist1, dist2, res),
    dist1/dist2 in original (unsorted) order, exact after host patch."""
    xyz1 = np.asarray(xyz1)
    xyz2 = np.asarray(xyz2)
    B, N, _ = xyz1.shape
    M = xyz2.shape[1]
    W = BAND_W
    halves = N_CORES // B  # row-halves per batch (2)
    n_rows = N // halves
    C = n_rows + W
    nc = _get_program(n_rows, W)

    ident = np.eye(128, dtype=np.float16)

    # host prep: radius sort, augmented operands, per-core shifted rhs
    perms1, perms2 = [], []
    a_s, c_s, ra_s, rc_s = [], [], [], []
    rhs_aug = []
    for b in range(B):
        r1 = np.linalg.norm(xyz1[b].astype(np.float64), axis=1)
        r2 = np.linalg.norm(xyz2[b].astype(np.float64), axis=1)
        p1 = np.argsort(r1, kind="stable")
        p2 = np.argsort(r2, kind="stable")
        perms1.append(p1); perms2.append(p2)
        a_s.append(xyz1[b][p1]); c_s.append(xyz2[b][p2])
        ra_s.append(r1[p1]); rc_s.append(r2[p2])
        rhs_aug.append(_aug_rows(c_s[b], "rhs"))

    # padding column (far away): contributes 4*(sh+sl) = PAD_DIST
    pad_col = np.zeros((K_AUG,), np.float16)
    pad_col[9] = np.float16(4.0)
    pad_col[10] = np.float16(4.0)
    pad_col[11] = np.float16(PAD_DIST / 4.0)
    pad_col[12] = np.float16(0.0)

    in_maps = []
    bases = []
    for c in range(N_CORES):
        b, h = divmod(c, halves)
        lhsT = _aug_rows(a_s[b][h * n_rows:(h + 1) * n_rows], "lhs")
        base = h * n_rows + 64 - W // 2  # global col of local col 0
        bases.append(base)
        rhs = np.repeat(pad_col[:, None], C, axis=1).astype(np.float16)
        g0, g1 = max(base, 0), min(base + C, M)
        rhs[:, g0 - base:g1 - base] = rhs_aug[b][:, g0:g1]
        in_maps.append({"lhsT": lhsT, "rhs": rhs, "ident": ident})

    res = run_bass_kernel_spmd(nc, in_maps, list(range(N_CORES)), trace=trace)

    ROWT = n_rows // 128
    NB = C // 128
    dist1_s = np.empty((B, N), np.float64)
    dist2_s = np.full((B, M), np.inf, np.float64)
    for c in range(N_CORES):
        b, h = divmod(c, halves)
        o1 = res.results[c]["out1"].astype(np.float64)  # [128, ROWT]
        o2 = res.results[c]["out2"].astype(np.float64)  # [128, NB]
        dist1_s[b, h * n_rows:(h + 1) * n_rows] = o1.T.reshape(-1)
        cols = bases[c] + np.arange(NB * 128)
        valid = (cols >= 0) & (cols < M)
        np.minimum.at(dist2_s[b], cols[valid], o2.T.reshape(-1)[valid])
    dist1_s /= DSCALE
    dist2_s /= DSCALE

    # --- exact host patch for at-risk points -----------------------------
    # Window of global tile T (row range [128T,128T+128)): global cols
    # [64 - W/2 + 128T, 64 + W/2 + 128T) intersected with [0, M).
    nt = N // 128
    w_lo = np.maximum(64 - W // 2 + 128 * np.arange(nt), 0)
    w_hi = np.minimum(64 + W // 2 + 128 * np.arange(nt), M)
    for b in range(B):
        ra, rc = ra_s[b], rc_s[b]
        a, cc = a_s[b], c_s[b]
        # coverage radius per sorted row: window covers rc[w_lo[t]..w_hi[t]-1]
        cov1 = np.empty(N)
        for t in range(nt):
            lo = -np.inf if w_lo[t] == 0 else rc[w_lo[t]]
            hi = np.inf if w_hi[t] == M else rc[w_hi[t] - 1]
            rr = ra[128 * t:128 * (t + 1)]
            cov1[128 * t:128 * (t + 1)] = np.minimum(rr - lo, hi - rr)
        # coverage per sorted col: tiles t with w_lo[t] <= j < w_hi[t] form a
        # contiguous range; their rows span a contiguous sorted-row range.
        j_all = np.arange(M)
        ft = np.searchsorted(w_hi - 1, j_all, "left")   # first tile covering j
        lt = np.searchsorted(w_lo, j_all, "right") - 1  # last tile covering j
        lo_r = np.where(ft <= 0, -np.inf, ra[np.minimum(ft * 128, N - 1)])
        hi_r = np.where(lt >= nt - 1, np.inf, ra[np.minimum((lt + 1) * 128 - 1, N - 1)])
        cov2 = np.minimum(rc - lo_r, hi_r - rc)
        risk1 = np.where(dist1_s[b] > cov1 ** 2 * 0.997 - 1e-6)[0]
        risk2 = np.where(dist2_s[b] > cov2 ** 2 * 0.997 - 1e-6)[0]
        if len(risk1):
            d = ((a[risk1][:, None, :].astype(np.float64)
                  - cc[None, :, :].astype(np.float64)) ** 2).sum(-1)
            dist1_s[b][risk1] = d.min(1)
        if len(risk2):
            d = ((cc[risk2][:, None, :].astype(np.float64)
                  - a[None, :, :].astype(np.float64)) ** 2).sum(-1)
            dist2_s[b][risk2] = d.min(1)

    # unsort back to original order
    dist1 = np.empty_like(dist1_s)
    dist2 = np.empty_like(dist2_s)
    for b in range(B):
        dist1[b][perms1[b]] = dist1_s[b]
        dist2[b][perms2[b]] = dist2_s[b]
    return dist1, dist2, res


def _finalize(dist1, dist2):
    dist1 = np.maximum(dist1, 0.0)
    dist2 = np.maximum(dist2, 0.0)
    cd_p = (np.sqrt(dist1).mean(axis=1) + np.sqrt(dist2).mean(axis=1)) / 2.0
    cd_t = dist1.mean(axis=1) + dist2.mean(axis=1)
    p1 = (dist1 < F1_THRESHOLD).mean(axis=1)
    p2 = (dist2 < F1_THRESHOLD).mean(axis=1)
    denom = p1 + p2
    f1 = np.where(denom > 0, 2.0 * p1 * p2 / np.where(denom > 0, denom, 1.0), 0.0)
    return (cd_p.astype(np.float32), cd_t.astype(np.float32),
            f1.astype(np.float32))


def kernel(xyz1, xyz2):
    dist1, dist2, _ = run_device(xyz1, xyz2, trace=False)
    return _finalize(dist1, dist2)
